# revision 12
# baseline (speedup 1.0000x reference)
"""GNN message-passing (scatter-mean + 2-layer node MLP) on 8 TRN2 NeuronCores.

Problem (fixed shapes):
    x [50000,128] f32, edge_index [2,800000] i64, edge_attr [800000,128] f32,
    W1 [256,256], b1 [256], W2 [256,128], b2 [128]
    out[n] = relu(concat(x[n], mean_{e: dst[e]=n} edge_attr[e]) @ W1 + b1) @ W2 + b2

Sharding: edges are partitioned by destination-node range (graph partitioning),
so each of the 8 cores owns 6250 nodes and exactly the edges that point at
them — no cross-core collectives at all.  Host-side preprocessing sorts edges
by destination and lays them out in an SBUF-friendly padded format; the device
does all the heavy math (segment sums over 410 MB of edge features + MLP).

Device algorithm per core (feature-major everywhere):
  for each 256-node window (25 windows x 256 = 6400 padded nodes):
    for each 128-edge chunk of the window (padded, dst=-1 for pads):
      onehot[e, j] = (dst_local[e] == j)            (DVE tensor_scalar is_equal)
      sums_psum[f, j] += attr_chunk[e, f]^T @ onehot  (PE, float32r, N=256)
    agg = sums_psum * recip_counts[window]            (DVE)
    h_i = relu(W1[0:128, i]^T @ xT_win + W1[128:256, i]^T @ agg + b1_i)
    out = W2[0:128]^T @ h_0 + W2[128:256]^T @ h_1 + b2
    DMA out -> outT[:, window]
Output is produced feature-major [128, 6400] per core; host transposes back.
"""

import os

import numpy as np

import concourse.bass as bass
import concourse.mybir as mybir
import concourse.tile as tile
from concourse.bass_utils import run_bass_kernel_spmd

# ----------------------------------------------------------------------------
# Workaround: this walrus build allows at most 1 sync-wait per instruction
# (any engine template).  Tile can attach several waits to one instruction, so
# after tracing we rewrite the BIR: for each instruction carrying k>1 waits,
# insert k-1 same-engine NoOps right before it, each carrying one wait.  The
# engine executes in order, so it stalls at the NoOps instead of at the
# instruction itself -- the set of satisfied conditions before the instruction
# executes is unchanged.
# ----------------------------------------------------------------------------
_MAX_WAITS = 1


def _spread_waits(nc):
    counter = [0]
    for f in nc.m.functions:
        for bb in f.blocks:
            insts = list(bb.instructions)
            new = []
            changed = False
            for inst in insts:
                si = getattr(inst, "sync_info", None)
                waits = list(si.on_wait) if si is not None else []
                if len(waits) > _MAX_WAITS:
                    spill, keep = waits[:-_MAX_WAITS], waits[-_MAX_WAITS:]
                    for wsub in spill:
                        nop = mybir.InstNoOp(
                            name=f"I-waitspread-{counter[0]}", ins=[], outs=[]
                        )
                        counter[0] += 1
                        nop.engine = inst.engine
                        nop.sync_info = mybir.SyncInfo(on_wait=[wsub], on_update=[])
                        new.append(nop)
                    inst.sync_info = mybir.SyncInfo(
                        on_wait=keep, on_update=list(si.on_update)
                    )
                    changed = True
                new.append(inst)
            if changed:
                bb.instructions = new


# ----------------------------------------------------------------------------
# Problem constants
# ----------------------------------------------------------------------------
N_NODES = 50000
N_EDGES = 800000
F = 128  # edge/node feature dim
HID = 256
F_OUT = 128
M = 8  # cores
NPC = N_NODES // M  # 6250 nodes per core
WN = 256  # nodes per MLP window
NW = (NPC + WN - 1) // WN  # 25 MLP windows
WN1 = 128  # nodes per phase-1 (segment-sum) window
NW1 = (NPC + WN1 - 1) // WN1  # 50 phase-1 windows
NPAD = NW * WN  # 6400
PHASE1_BF16 = True  # ship edge features as bf16 (half DMA, ~5e-4 rel err)
KB = 9  # onehot chunks built per DVE tensor_tensor op
ACT_FRAC = 0.25  # fraction of onehot builds offloaded to the Scalar engine
FC = F + 1  # interleaved chunk stride: 128 attr cols + 1 dst col
P = 128

F32 = mybir.dt.float32
F32R = mybir.dt.float32r
BF16 = mybir.dt.bfloat16

# Last kernel-run results, for test harnesses (exec_time_ns etc).
LAST_RUN = None

_BUILD_CACHE = {}


def _build_device_kernel(cw, spread=True):
    """Build the SPMD Bass program for per-phase1-window chunk counts `cw`.

    Phase-1 windows have WP nodes (WN1=128 in bf16 mode, WN=256 in f32r
    mode); HALVES of them make up one 256-node MLP window.
    """
    C = int(sum(cw))
    WP = WN1 if PHASE1_BF16 else WN
    HALVES = WN // WP
    ADT = BF16 if PHASE1_BF16 else F32R  # attr + onehot dtype
    DDT = BF16 if PHASE1_BF16 else F32  # dst-index dtype

    nc = bass.Bass("TRN2")

    attr_d = nc.declare_dram_parameter("attr", [P, C * FC], ADT, isOutput=False)
    xt_d = nc.declare_dram_parameter("xt", [P, NPAD], F32R, isOutput=False)
    recip_d = nc.declare_dram_parameter("recip", [NPAD], F32, isOutput=False)
    w1_d = nc.declare_dram_parameter("w1", [2 * P, HID], F32R, isOutput=False)
    b1_d = nc.declare_dram_parameter("b1", [P, 2], F32, isOutput=False)
    w2_d = nc.declare_dram_parameter("w2", [2 * P, F_OUT], F32R, isOutput=False)
    b2_d = nc.declare_dram_parameter("b2", [P, 1], F32, isOutput=False)
    out_d = nc.declare_dram_parameter("outT", [P, NPAD], F32, isOutput=True)

    with tile.TileContext(nc) as tc:
        with (
            tc.tile_pool(name="const", bufs=1) as const,
            tc.tile_pool(name="attr", bufs=3) as attr_p,
            tc.tile_pool(name="oh", bufs=6) as oh_p,
            tc.tile_pool(name="oha", bufs=4) as oha_p,
            tc.tile_pool(name="abs", bufs=4) as abs_p,
            tc.tile_pool(name="xtw", bufs=3) as xt_p,
            tc.tile_pool(name="rcw", bufs=3) as rc_p,
            tc.tile_pool(name="agg", bufs=2) as agg_p,
            tc.tile_pool(name="hsb", bufs=4) as h_p,
            tc.tile_pool(name="osb", bufs=2) as o_p,
            tc.tile_pool(name="ps_s", bufs=2, space="PSUM") as ps_s,
            tc.tile_pool(name="ps_h", bufs=4, space="PSUM") as ps_h,
            tc.tile_pool(name="ps_o", bufs=2, space="PSUM") as ps_o,
        ):
            # --- constants ---
            iota_i = const.tile([P, WP], mybir.dt.int32)
            nc.gpsimd.iota(iota_i[:], pattern=[[1, WP]], base=0, channel_multiplier=0)
            iota_r = const.tile([P, KB, WP], DDT)  # 0..WP-1 per lane, x KB
            for j in range(KB):
                nc.vector.tensor_copy(out=iota_r[:, j, :], in_=iota_i[:])

            r_ap = recip_d[:]

            w1_sb = const.tile([P, 2, HID], F32R)  # [p, k, h]: W1[k*128+p, h]
            nc.sync.dma_start(
                out=w1_sb[:], in_=w1_d[:].rearrange("(k p) h -> p k h", p=P)
            )
            w2_sb = const.tile([P, 2, F_OUT], F32R)
            nc.sync.dma_start(
                out=w2_sb[:], in_=w2_d[:].rearrange("(k p) h -> p k h", p=P)
            )
            b1_sb = const.tile([P, 2], F32)
            nc.sync.dma_start(out=b1_sb[:], in_=b1_d[:])
            b2_sb = const.tile([P, 1], F32)
            nc.sync.dma_start(out=b2_sb[:], in_=b2_d[:])

            # --- per-MLP-window pipeline ---
            off = 0
            chunk_no = 0
            for w in range(NW):
                nsl = slice(w * WN, (w + 1) * WN)
                agg = agg_p.tile([P, WN], F32R)

                xt_w = xt_p.tile([P, WN], F32R, tag="xtw")
                nc.sync.dma_start(out=xt_w[:], in_=xt_d[:, nsl])
                rc_w = rc_p.tile([P, WN], F32, tag="rcw")
                rc_src = bass.AP(
                    tensor=r_ap.tensor,
                    offset=r_ap.offset + w * WN,
                    ap=[[0, P], [1, WN]],
                )
                nc.sync.dma_start(out=rc_w[:], in_=rc_src)

                for hw in range(HALVES):
                    wp = w * HALVES + hw
                    cwn = cw[wp]
                    asl = slice(hw * WP, (hw + 1) * WP)
                    psl = slice(wp * WP, (wp + 1) * WP)
                    if cwn == 0:
                        nc.vector.memset(agg[:, asl], 0.0)
                        continue

                    a_t = attr_p.tile([P, cwn * FC], ADT, tag="attr")
                    nc.sync.dma_start(
                        out=a_t[:], in_=attr_d[:, off * FC : (off + cwn) * FC]
                    )
                    a_v = a_t[:].rearrange("p (c f) -> p c f", f=FC)

                    sums = ps_s.tile([P, WP], F32, tag="sums")
                    ndve = cwn - int(cwn * ACT_FRAC)
                    for c0 in range(0, ndve, KB):
                        kb = min(KB, ndve - c0)
                        oh = oh_p.tile([P, KB, WP], ADT, tag="oh")
                        d_sl = a_v[:, c0 : c0 + kb, F : F + 1]
                        d_bc = bass.AP(
                            tensor=d_sl.tensor,
                            offset=d_sl.offset,
                            ap=list(d_sl.ap[:2]) + [[0, WP]],
                        )
                        nc.vector.tensor_tensor(
                            out=oh[:, :kb, :],
                            in0=iota_r[:, :kb, :],
                            in1=d_bc,
                            op=mybir.AluOpType.is_equal,
                        )
                        for j in range(kb):
                            c = c0 + j
                            nc.tensor.matmul(
                                out=sums[:],
                                lhsT=a_v[:, c, 0:F],
                                rhs=oh[:, j, :],
                                start=(c == 0),
                                stop=(c == cwn - 1),
                            )
                    for c in range(ndve, cwn):
                        # onehot on the Scalar engine: relu(1 - |d - iota|)
                        t_t = abs_p.tile([P, WP], ADT, tag="abs")
                        nc.scalar.activation(
                            out=t_t[:],
                            in_=iota_r[:, 0, :],
                            func=mybir.ActivationFunctionType.Abs,
                            bias=a_v[:, c, F : F + 1],
                            scale=-1.0,
                        )
                        oh1 = oha_p.tile([P, WP], ADT, tag="oha")
                        nc.scalar.activation(
                            out=oh1[:],
                            in_=t_t[:],
                            func=mybir.ActivationFunctionType.Relu,
                            bias=1.0,
                            scale=-1.0,
                        )
                        nc.tensor.matmul(
                            out=sums[:],
                            lhsT=a_v[:, c, 0:F],
                            rhs=oh1[:],
                            start=(c == 0),
                            stop=(c == cwn - 1),
                        )
                    # agg = sums / max(count, 1)
                    nc.vector.tensor_mul(
                        out=agg[:, asl], in0=sums[:], in1=rc_w[:, asl]
                    )
                    off += cwn

                # --- node MLP (feature-major, nodes on free dim) ---
                h_sbs = []
                for hi in range(2):
                    h_ps = ps_h.tile([P, WN], F32, tag="h")
                    hsl = slice(hi * P, (hi + 1) * P)
                    nc.tensor.matmul(
                        out=h_ps[:],
                        lhsT=w1_sb[:, 0, hsl],
                        rhs=xt_w[:],
                        start=True,
                        stop=False,
                    )
                    nc.tensor.matmul(
                        out=h_ps[:],
                        lhsT=w1_sb[:, 1, hsl],
                        rhs=agg[:],
                        start=False,
                        stop=True,
                    )
                    h_sb = h_p.tile([P, WN], F32R, tag="hsb")
                    nc.scalar.activation(
                        out=h_sb[:],
                        in_=h_ps[:],
                        func=mybir.ActivationFunctionType.Relu,
                        bias=b1_sb[:, hi : hi + 1],
                        scale=1.0,
                    )
                    h_sbs.append(h_sb)

                o_ps = ps_o.tile([P, WN], F32, tag="o")
                for k in range(2):
                    nc.tensor.matmul(
                        out=o_ps[:],
                        lhsT=w2_sb[:, k, :],
                        rhs=h_sbs[k][:],
                        start=(k == 0),
                        stop=(k == 1),
                    )
                o_sb = o_p.tile([P, WN], F32, tag="osb")
                nc.vector.tensor_scalar_add(
                    out=o_sb[:], in0=o_ps[:], scalar1=b2_sb[:, 0:1]
                )
                nc.sync.dma_start(out=out_d[:, nsl], in_=o_sb[:])

    if spread:
        _spread_waits(nc)
    return nc


def _preprocess(x, edge_index, edge_attr):
    """Sort edges by dst, bucket into per-core node-range shards, pad each
    phase-1 window's edge list to a multiple of 128 (chunk counts shared
    across cores so one SPMD program fits all)."""
    import ml_dtypes

    WP = WN1 if PHASE1_BF16 else WN
    NWP = NPAD // WP
    adt = ml_dtypes.bfloat16 if PHASE1_BF16 else np.float32

    dst = np.asarray(edge_index[1], dtype=np.int64)
    order = np.argsort(dst, kind="stable")
    ds = dst[order]

    # phase-1 window boundaries in the sorted edge array: [M, NWP+1]
    los = np.empty((M, NWP + 1), dtype=np.int64)
    for c in range(M):
        for w in range(NWP + 1):
            los[c, w] = c * NPC + min(w * WP, NPC)
    bounds = np.searchsorted(ds, los.ravel()).reshape(M, NWP + 1)
    L = bounds[:, 1:] - bounds[:, :-1]  # [M, NWP]
    cw = tuple(max(int(v), 1) for v in np.ceil(L.max(axis=0) / P).astype(np.int64))
    C = int(sum(cw))

    counts = np.bincount(dst, minlength=N_NODES).astype(np.float32)
    recip_full = 1.0 / np.maximum(counts, 1.0)

    per_core = []
    for c in range(M):
        idx = np.zeros(C * P, dtype=np.int64)
        dloc = np.full(C * P, -1.0, dtype=np.float32)
        pos = 0
        for w in range(NWP):
            s, e = int(bounds[c, w]), int(bounds[c, w + 1])
            n = e - s
            idx[pos : pos + n] = order[s:e]
            dloc[pos : pos + n] = (ds[s:e] - (c * NPC + w * WP)).astype(np.float32)
            pos += cw[w] * P
        attr = np.empty((C * P, F + 1), dtype=adt)  # last col = local dst idx
        attr[:, :F] = edge_attr[idx].astype(adt)
        attr[:, F] = dloc.astype(adt)
        attr_t = np.ascontiguousarray(
            attr.reshape(C, P, F + 1).transpose(1, 0, 2).reshape(P, C * (F + 1))
        )

        xt = np.zeros((P, NPAD), dtype=np.float32)
        xt[:, :NPC] = x[c * NPC : (c + 1) * NPC].T
        recip = np.zeros(NPAD, dtype=np.float32)
        recip[:NPC] = recip_full[c * NPC : (c + 1) * NPC]
        per_core.append({"attr": attr_t, "xt": xt, "recip": recip})
    return cw, per_core


def kernel(x, edge_index, edge_attr, W1, b1, W2, b2):
    global LAST_RUN
    x = np.ascontiguousarray(np.asarray(x, dtype=np.float32))
    edge_attr = np.ascontiguousarray(np.asarray(edge_attr, dtype=np.float32))
    W1 = np.ascontiguousarray(np.asarray(W1, dtype=np.float32))
    W2 = np.ascontiguousarray(np.asarray(W2, dtype=np.float32))
    b1c = np.ascontiguousarray(
        np.asarray(b1, dtype=np.float32).reshape(2, P).T
    )  # [128, 2]
    b2c = np.ascontiguousarray(np.asarray(b2, dtype=np.float32).reshape(P, 1))

    cw, per_core = _preprocess(x, edge_index, edge_attr)

    if cw not in _BUILD_CACHE:
        _BUILD_CACHE[cw] = _build_device_kernel(cw)
    nc = _BUILD_CACHE[cw]

    in_maps = []
    for c in range(M):
        pc = per_core[c]
        in_maps.append(
            {
                "attr": pc["attr"],
                "xt": pc["xt"],
                "recip": pc["recip"],
                "w1": W1,
                "b1": b1c,
                "w2": W2,
                "b2": b2c,
            }
        )

    trace = os.environ.get("KERNEL_TRACE") == "1"
    res = run_bass_kernel_spmd(nc, in_maps, core_ids=list(range(M)), trace=trace)
    LAST_RUN = res

    out = np.empty((N_NODES, F_OUT), dtype=np.float32)
    for c in range(M):
        out[c * NPC : (c + 1) * NPC] = res.results[c]["outT"][:, :NPC].T
    return out


# revision 13
# speedup vs baseline: 1.0509x; 1.0509x over previous
"""GNN message-passing (scatter-mean + 2-layer node MLP) on 8 TRN2 NeuronCores.

Problem (fixed shapes):
    x [50000,128] f32, edge_index [2,800000] i64, edge_attr [800000,128] f32,
    W1 [256,256], b1 [256], W2 [256,128], b2 [128]
    out[n] = relu(concat(x[n], mean_{e: dst[e]=n} edge_attr[e]) @ W1 + b1) @ W2 + b2

Sharding: edges are partitioned by destination-node range (graph partitioning),
so each of the 8 cores owns 6250 nodes and exactly the edges that point at
them — no cross-core collectives at all.  Host-side preprocessing sorts edges
by destination and lays them out in an SBUF-friendly padded format; the device
does all the heavy math (segment sums over 410 MB of edge features + MLP).

Device algorithm per core (feature-major everywhere):
  for each 256-node window (25 windows x 256 = 6400 padded nodes):
    for each 128-edge chunk of the window (padded, dst=-1 for pads):
      onehot[e, j] = (dst_local[e] == j)            (DVE tensor_scalar is_equal)
      sums_psum[f, j] += attr_chunk[e, f]^T @ onehot  (PE, float32r, N=256)
    agg = sums_psum * recip_counts[window]            (DVE)
    h_i = relu(W1[0:128, i]^T @ xT_win + W1[128:256, i]^T @ agg + b1_i)
    out = W2[0:128]^T @ h_0 + W2[128:256]^T @ h_1 + b2
    DMA out -> outT[:, window]
Output is produced feature-major [128, 6400] per core; host transposes back.
"""

import os

import numpy as np

import concourse.bass as bass
import concourse.mybir as mybir
import concourse.tile as tile
from concourse.bass_utils import run_bass_kernel_spmd

# ----------------------------------------------------------------------------
# Workaround: this walrus build allows at most 1 sync-wait per instruction
# (any engine template).  Tile can attach several waits to one instruction, so
# after tracing we rewrite the BIR: for each instruction carrying k>1 waits,
# insert k-1 same-engine NoOps right before it, each carrying one wait.  The
# engine executes in order, so it stalls at the NoOps instead of at the
# instruction itself -- the set of satisfied conditions before the instruction
# executes is unchanged.
# ----------------------------------------------------------------------------
_MAX_WAITS = 1


def _spread_waits(nc):
    counter = [0]
    for f in nc.m.functions:
        for bb in f.blocks:
            insts = list(bb.instructions)
            new = []
            changed = False
            for inst in insts:
                si = getattr(inst, "sync_info", None)
                waits = list(si.on_wait) if si is not None else []
                if len(waits) > _MAX_WAITS:
                    spill, keep = waits[:-_MAX_WAITS], waits[-_MAX_WAITS:]
                    for wsub in spill:
                        nop = mybir.InstNoOp(
                            name=f"I-waitspread-{counter[0]}", ins=[], outs=[]
                        )
                        counter[0] += 1
                        nop.engine = inst.engine
                        nop.sync_info = mybir.SyncInfo(on_wait=[wsub], on_update=[])
                        new.append(nop)
                    inst.sync_info = mybir.SyncInfo(
                        on_wait=keep, on_update=list(si.on_update)
                    )
                    changed = True
                new.append(inst)
            if changed:
                bb.instructions = new


# ----------------------------------------------------------------------------
# Problem constants
# ----------------------------------------------------------------------------
N_NODES = 50000
N_EDGES = 800000
F = 128  # edge/node feature dim
HID = 256
F_OUT = 128
M = 8  # cores
NPC = N_NODES // M  # 6250 nodes per core
WN = 256  # nodes per MLP window
NW = (NPC + WN - 1) // WN  # 25 MLP windows
WN1 = 128  # nodes per phase-1 (segment-sum) window
NW1 = (NPC + WN1 - 1) // WN1  # 50 phase-1 windows
NPAD = NW * WN  # 6400
PHASE1_BF16 = True  # ship edge features as bf16 (half DMA, ~5e-4 rel err)
KB = 9  # onehot chunks built per DVE tensor_tensor op
ACT_FRAC = 0.18  # fraction of onehot builds offloaded to the Scalar engine
FC = F + 1  # interleaved chunk stride: 128 attr cols + 1 dst col
P = 128

F32 = mybir.dt.float32
F32R = mybir.dt.float32r
BF16 = mybir.dt.bfloat16

# Last kernel-run results, for test harnesses (exec_time_ns etc).
LAST_RUN = None

_BUILD_CACHE = {}


def _build_device_kernel(cw, spread=True):
    """Build the SPMD Bass program for per-phase1-window chunk counts `cw`.

    Phase-1 windows have WP nodes (WN1=128 in bf16 mode, WN=256 in f32r
    mode); HALVES of them make up one 256-node MLP window.
    """
    C = int(sum(cw))
    WP = WN1 if PHASE1_BF16 else WN
    HALVES = WN // WP
    ADT = BF16 if PHASE1_BF16 else F32R  # attr + onehot dtype
    DDT = BF16 if PHASE1_BF16 else F32  # dst-index dtype

    nc = bass.Bass("TRN2")

    attr_d = nc.declare_dram_parameter("attr", [P, C * FC], ADT, isOutput=False)
    xt_d = nc.declare_dram_parameter("xt", [P, NPAD], F32R, isOutput=False)
    recip_d = nc.declare_dram_parameter("recip", [NPAD], F32, isOutput=False)
    w1_d = nc.declare_dram_parameter("w1", [2 * P, HID], F32R, isOutput=False)
    b1_d = nc.declare_dram_parameter("b1", [P, 2], F32, isOutput=False)
    w2_d = nc.declare_dram_parameter("w2", [2 * P, F_OUT], F32R, isOutput=False)
    b2_d = nc.declare_dram_parameter("b2", [P, 1], F32, isOutput=False)
    out_d = nc.declare_dram_parameter("outT", [P, NPAD], F32, isOutput=True)

    with tile.TileContext(nc) as tc:
        with (
            tc.tile_pool(name="const", bufs=1) as const,
            tc.tile_pool(name="attr", bufs=3) as attr_p,
            tc.tile_pool(name="oh", bufs=6) as oh_p,
            tc.tile_pool(name="oha", bufs=8) as oha_p,
            tc.tile_pool(name="abs", bufs=4) as abs_p,
            tc.tile_pool(name="xtw", bufs=3) as xt_p,
            tc.tile_pool(name="rcw", bufs=3) as rc_p,
            tc.tile_pool(name="agg", bufs=3) as agg_p,
            tc.tile_pool(name="hsb", bufs=4) as h_p,
            tc.tile_pool(name="osb", bufs=2) as o_p,
            tc.tile_pool(name="ps_s", bufs=2, space="PSUM") as ps_s,
            tc.tile_pool(name="ps_h", bufs=4, space="PSUM") as ps_h,
            tc.tile_pool(name="ps_o", bufs=2, space="PSUM") as ps_o,
        ):
            # --- constants ---
            iota_i = const.tile([P, WP], mybir.dt.int32)
            nc.gpsimd.iota(iota_i[:], pattern=[[1, WP]], base=0, channel_multiplier=0)
            iota_r = const.tile([P, KB, WP], DDT)  # 0..WP-1 per lane, x KB
            for j in range(KB):
                nc.vector.tensor_copy(out=iota_r[:, j, :], in_=iota_i[:])

            r_ap = recip_d[:]

            w1_sb = const.tile([P, 2, HID], F32R)  # [p, k, h]: W1[k*128+p, h]
            nc.sync.dma_start(
                out=w1_sb[:], in_=w1_d[:].rearrange("(k p) h -> p k h", p=P)
            )
            w2_sb = const.tile([P, 2, F_OUT], F32R)
            nc.sync.dma_start(
                out=w2_sb[:], in_=w2_d[:].rearrange("(k p) h -> p k h", p=P)
            )
            b1_sb = const.tile([P, 2], F32)
            nc.sync.dma_start(out=b1_sb[:], in_=b1_d[:])
            b2_sb = const.tile([P, 1], F32)
            nc.sync.dma_start(out=b2_sb[:], in_=b2_d[:])

            # --- per-MLP-window pipeline (MLP delayed one window) ---
            off = 0
            chunk_no = 0
            pending = []
            for w in range(list(range(NW))[-1] + 1 + 1):
                if w < NW:
                    pass
                else:
                    # flush: emit final window's MLP
                    w_m, agg, xt_w = pending.pop(0)
                    nsl = slice(w_m * WN, (w_m + 1) * WN)
                    h_sbs = []
                    for hi in range(2):
                        h_ps = ps_h.tile([P, WN], F32, tag="h")
                        hsl = slice(hi * P, (hi + 1) * P)
                        nc.tensor.matmul(
                            out=h_ps[:],
                            lhsT=w1_sb[:, 0, hsl],
                            rhs=xt_w[:],
                            start=True,
                            stop=False,
                        )
                        nc.tensor.matmul(
                            out=h_ps[:],
                            lhsT=w1_sb[:, 1, hsl],
                            rhs=agg[:],
                            start=False,
                            stop=True,
                        )
                        h_sb = h_p.tile([P, WN], F32R, tag="hsb")
                        nc.scalar.activation(
                            out=h_sb[:],
                            in_=h_ps[:],
                            func=mybir.ActivationFunctionType.Relu,
                            bias=b1_sb[:, hi : hi + 1],
                            scale=1.0,
                        )
                        h_sbs.append(h_sb)
                    o_ps = ps_o.tile([P, WN], F32, tag="o")
                    for k in range(2):
                        nc.tensor.matmul(
                            out=o_ps[:],
                            lhsT=w2_sb[:, k, :],
                            rhs=h_sbs[k][:],
                            start=(k == 0),
                            stop=(k == 1),
                        )
                    o_sb = o_p.tile([P, WN], F32, tag="osb")
                    nc.vector.tensor_scalar_add(
                        out=o_sb[:], in0=o_ps[:], scalar1=b2_sb[:, 0:1]
                    )
                    nc.sync.dma_start(out=out_d[:, nsl], in_=o_sb[:])
                    break
                nsl = slice(w * WN, (w + 1) * WN)
                agg = agg_p.tile([P, WN], F32R)

                xt_w = xt_p.tile([P, WN], F32R, tag="xtw")
                nc.sync.dma_start(out=xt_w[:], in_=xt_d[:, nsl])
                rc_w = rc_p.tile([P, WN], F32, tag="rcw")
                rc_src = bass.AP(
                    tensor=r_ap.tensor,
                    offset=r_ap.offset + w * WN,
                    ap=[[0, P], [1, WN]],
                )
                nc.sync.dma_start(out=rc_w[:], in_=rc_src)

                for hw in range(HALVES):
                    wp = w * HALVES + hw
                    cwn = cw[wp]
                    asl = slice(hw * WP, (hw + 1) * WP)
                    psl = slice(wp * WP, (wp + 1) * WP)
                    if cwn == 0:
                        nc.vector.memset(agg[:, asl], 0.0)
                        continue

                    a_t = attr_p.tile([P, cwn * FC], ADT, tag="attr")
                    nc.sync.dma_start(
                        out=a_t[:], in_=attr_d[:, off * FC : (off + cwn) * FC]
                    )
                    a_v = a_t[:].rearrange("p (c f) -> p c f", f=FC)

                    sums = ps_s.tile([P, WP], F32, tag="sums")
                    nact = int(cwn * ACT_FRAC)
                    for c in range(nact):
                        # onehot on the Scalar engine: relu(1 - |d - iota|)
                        t_t = abs_p.tile([P, WP], ADT, tag="abs")
                        nc.scalar.activation(
                            out=t_t[:],
                            in_=iota_r[:, 0, :],
                            func=mybir.ActivationFunctionType.Abs,
                            bias=a_v[:, c, F : F + 1],
                            scale=-1.0,
                        )
                        oh1 = oha_p.tile([P, WP], ADT, tag="oha")
                        nc.scalar.activation(
                            out=oh1[:],
                            in_=t_t[:],
                            func=mybir.ActivationFunctionType.Relu,
                            bias=1.0,
                            scale=-1.0,
                        )
                        nc.tensor.matmul(
                            out=sums[:],
                            lhsT=a_v[:, c, 0:F],
                            rhs=oh1[:],
                            start=(c == 0),
                            stop=(c == cwn - 1),
                        )
                    for c0 in range(nact, cwn, KB):
                        kb = min(KB, cwn - c0)
                        oh = oh_p.tile([P, KB, WP], ADT, tag="oh")
                        d_sl = a_v[:, c0 : c0 + kb, F : F + 1]
                        d_bc = bass.AP(
                            tensor=d_sl.tensor,
                            offset=d_sl.offset,
                            ap=list(d_sl.ap[:2]) + [[0, WP]],
                        )
                        nc.vector.tensor_tensor(
                            out=oh[:, :kb, :],
                            in0=iota_r[:, :kb, :],
                            in1=d_bc,
                            op=mybir.AluOpType.is_equal,
                        )
                        for j in range(kb):
                            c = c0 + j
                            nc.tensor.matmul(
                                out=sums[:],
                                lhsT=a_v[:, c, 0:F],
                                rhs=oh[:, j, :],
                                start=(c == 0),
                                stop=(c == cwn - 1),
                            )
                    # agg = sums / max(count, 1)
                    nc.vector.tensor_mul(
                        out=agg[:, asl], in0=sums[:], in1=rc_w[:, asl]
                    )
                    off += cwn

                pending.append((w, agg, xt_w))
                if len(pending) < 2:
                    continue
                w_m, agg, xt_w = pending.pop(0)
                nsl = slice(w_m * WN, (w_m + 1) * WN)

                # --- node MLP (feature-major, nodes on free dim) ---
                h_sbs = []
                for hi in range(2):
                    h_ps = ps_h.tile([P, WN], F32, tag="h")
                    hsl = slice(hi * P, (hi + 1) * P)
                    nc.tensor.matmul(
                        out=h_ps[:],
                        lhsT=w1_sb[:, 0, hsl],
                        rhs=xt_w[:],
                        start=True,
                        stop=False,
                    )
                    nc.tensor.matmul(
                        out=h_ps[:],
                        lhsT=w1_sb[:, 1, hsl],
                        rhs=agg[:],
                        start=False,
                        stop=True,
                    )
                    h_sb = h_p.tile([P, WN], F32R, tag="hsb")
                    nc.scalar.activation(
                        out=h_sb[:],
                        in_=h_ps[:],
                        func=mybir.ActivationFunctionType.Relu,
                        bias=b1_sb[:, hi : hi + 1],
                        scale=1.0,
                    )
                    h_sbs.append(h_sb)

                o_ps = ps_o.tile([P, WN], F32, tag="o")
                for k in range(2):
                    nc.tensor.matmul(
                        out=o_ps[:],
                        lhsT=w2_sb[:, k, :],
                        rhs=h_sbs[k][:],
                        start=(k == 0),
                        stop=(k == 1),
                    )
                o_sb = o_p.tile([P, WN], F32, tag="osb")
                nc.vector.tensor_scalar_add(
                    out=o_sb[:], in0=o_ps[:], scalar1=b2_sb[:, 0:1]
                )
                nc.sync.dma_start(out=out_d[:, nsl], in_=o_sb[:])

    if spread:
        _spread_waits(nc)
    return nc


def _preprocess(x, edge_index, edge_attr):
    """Sort edges by dst, bucket into per-core node-range shards, pad each
    phase-1 window's edge list to a multiple of 128 (chunk counts shared
    across cores so one SPMD program fits all)."""
    import ml_dtypes

    WP = WN1 if PHASE1_BF16 else WN
    NWP = NPAD // WP
    adt = ml_dtypes.bfloat16 if PHASE1_BF16 else np.float32

    dst = np.asarray(edge_index[1], dtype=np.int64)
    order = np.argsort(dst, kind="stable")
    ds = dst[order]

    # phase-1 window boundaries in the sorted edge array: [M, NWP+1]
    los = np.empty((M, NWP + 1), dtype=np.int64)
    for c in range(M):
        for w in range(NWP + 1):
            los[c, w] = c * NPC + min(w * WP, NPC)
    bounds = np.searchsorted(ds, los.ravel()).reshape(M, NWP + 1)
    L = bounds[:, 1:] - bounds[:, :-1]  # [M, NWP]
    cw = tuple(max(int(v), 1) for v in np.ceil(L.max(axis=0) / P).astype(np.int64))
    C = int(sum(cw))

    counts = np.bincount(dst, minlength=N_NODES).astype(np.float32)
    recip_full = 1.0 / np.maximum(counts, 1.0)

    per_core = []
    for c in range(M):
        idx = np.zeros(C * P, dtype=np.int64)
        dloc = np.full(C * P, -1.0, dtype=np.float32)
        pos = 0
        for w in range(NWP):
            s, e = int(bounds[c, w]), int(bounds[c, w + 1])
            n = e - s
            idx[pos : pos + n] = order[s:e]
            dloc[pos : pos + n] = (ds[s:e] - (c * NPC + w * WP)).astype(np.float32)
            pos += cw[w] * P
        attr = np.empty((C * P, F + 1), dtype=adt)  # last col = local dst idx
        attr[:, :F] = edge_attr[idx].astype(adt)
        attr[:, F] = dloc.astype(adt)
        attr_t = np.ascontiguousarray(
            attr.reshape(C, P, F + 1).transpose(1, 0, 2).reshape(P, C * (F + 1))
        )

        xt = np.zeros((P, NPAD), dtype=np.float32)
        xt[:, :NPC] = x[c * NPC : (c + 1) * NPC].T
        recip = np.zeros(NPAD, dtype=np.float32)
        recip[:NPC] = recip_full[c * NPC : (c + 1) * NPC]
        per_core.append({"attr": attr_t, "xt": xt, "recip": recip})
    return cw, per_core


def kernel(x, edge_index, edge_attr, W1, b1, W2, b2):
    global LAST_RUN
    x = np.ascontiguousarray(np.asarray(x, dtype=np.float32))
    edge_attr = np.ascontiguousarray(np.asarray(edge_attr, dtype=np.float32))
    W1 = np.ascontiguousarray(np.asarray(W1, dtype=np.float32))
    W2 = np.ascontiguousarray(np.asarray(W2, dtype=np.float32))
    b1c = np.ascontiguousarray(
        np.asarray(b1, dtype=np.float32).reshape(2, P).T
    )  # [128, 2]
    b2c = np.ascontiguousarray(np.asarray(b2, dtype=np.float32).reshape(P, 1))

    cw, per_core = _preprocess(x, edge_index, edge_attr)

    if cw not in _BUILD_CACHE:
        _BUILD_CACHE[cw] = _build_device_kernel(cw)
    nc = _BUILD_CACHE[cw]

    in_maps = []
    for c in range(M):
        pc = per_core[c]
        in_maps.append(
            {
                "attr": pc["attr"],
                "xt": pc["xt"],
                "recip": pc["recip"],
                "w1": W1,
                "b1": b1c,
                "w2": W2,
                "b2": b2c,
            }
        )

    trace = os.environ.get("KERNEL_TRACE") == "1"
    res = run_bass_kernel_spmd(nc, in_maps, core_ids=list(range(M)), trace=trace)
    LAST_RUN = res

    out = np.empty((N_NODES, F_OUT), dtype=np.float32)
    for c in range(M):
        out[c * NPC : (c + 1) * NPC] = res.results[c]["outT"][:, :NPC].T
    return out


# revision 15
# speedup vs baseline: 1.1184x; 1.0642x over previous
"""GNN message-passing (scatter-mean + 2-layer node MLP) on 8 TRN2 NeuronCores.

Problem (fixed shapes):
    x [50000,128] f32, edge_index [2,800000] i64, edge_attr [800000,128] f32,
    W1 [256,256], b1 [256], W2 [256,128], b2 [128]
    out[n] = relu(concat(x[n], mean_{e: dst[e]=n} edge_attr[e]) @ W1 + b1) @ W2 + b2

Sharding: edges are partitioned by destination-node range (graph partitioning),
so each of the 8 cores owns 6250 nodes and exactly the edges that point at
them — no cross-core collectives at all.  Host-side preprocessing sorts edges
by destination and lays them out in an SBUF-friendly padded format; the device
does all the heavy math (segment sums over 410 MB of edge features + MLP).

Device algorithm per core (feature-major everywhere):
  for each 256-node window (25 windows x 256 = 6400 padded nodes):
    for each 128-edge chunk of the window (padded, dst=-1 for pads):
      onehot[e, j] = (dst_local[e] == j)            (DVE tensor_scalar is_equal)
      sums_psum[f, j] += attr_chunk[e, f]^T @ onehot  (PE, float32r, N=256)
    agg = sums_psum * recip_counts[window]            (DVE)
    h_i = relu(W1[0:128, i]^T @ xT_win + W1[128:256, i]^T @ agg + b1_i)
    out = W2[0:128]^T @ h_0 + W2[128:256]^T @ h_1 + b2
    DMA out -> outT[:, window]
Output is produced feature-major [128, 6400] per core; host transposes back.
"""

import os

import numpy as np

import concourse.bass as bass
import concourse.mybir as mybir
import concourse.tile as tile
from concourse.bass_utils import run_bass_kernel_spmd

# ----------------------------------------------------------------------------
# Workaround: this walrus build allows at most 1 sync-wait per instruction
# (any engine template).  Tile can attach several waits to one instruction, so
# after tracing we rewrite the BIR: for each instruction carrying k>1 waits,
# insert k-1 same-engine NoOps right before it, each carrying one wait.  The
# engine executes in order, so it stalls at the NoOps instead of at the
# instruction itself -- the set of satisfied conditions before the instruction
# executes is unchanged.
# ----------------------------------------------------------------------------
_MAX_WAITS = 1


def _spread_waits(nc):
    counter = [0]
    for f in nc.m.functions:
        for bb in f.blocks:
            insts = list(bb.instructions)
            new = []
            changed = False
            for inst in insts:
                si = getattr(inst, "sync_info", None)
                waits = list(si.on_wait) if si is not None else []
                if len(waits) > _MAX_WAITS:
                    spill, keep = waits[:-_MAX_WAITS], waits[-_MAX_WAITS:]
                    for wsub in spill:
                        nop = mybir.InstNoOp(
                            name=f"I-waitspread-{counter[0]}", ins=[], outs=[]
                        )
                        counter[0] += 1
                        nop.engine = inst.engine
                        nop.sync_info = mybir.SyncInfo(on_wait=[wsub], on_update=[])
                        new.append(nop)
                    inst.sync_info = mybir.SyncInfo(
                        on_wait=keep, on_update=list(si.on_update)
                    )
                    changed = True
                new.append(inst)
            if changed:
                bb.instructions = new


# ----------------------------------------------------------------------------
# Problem constants
# ----------------------------------------------------------------------------
N_NODES = 50000
N_EDGES = 800000
F = 128  # edge/node feature dim
HID = 256
F_OUT = 128
M = 8  # cores
NPC = N_NODES // M  # 6250 nodes per core
WN = 256  # nodes per MLP window
NW = (NPC + WN - 1) // WN  # 25 MLP windows
WN1 = 128  # nodes per phase-1 (segment-sum) window
NW1 = (NPC + WN1 - 1) // WN1  # 50 phase-1 windows
NPAD = NW * WN  # 6400
PHASE1_BF16 = True  # ship edge features as bf16 (half DMA, ~5e-4 rel err)
KB = 9  # onehot chunks built per DVE tensor_tensor op
ACT_FRAC = 0.18  # fraction of onehot builds offloaded to the Scalar engine
FC = F + 1  # interleaved chunk stride: 128 attr cols + 1 dst col
P = 128

F32 = mybir.dt.float32
F32R = mybir.dt.float32r
BF16 = mybir.dt.bfloat16

# Last kernel-run results, for test harnesses (exec_time_ns etc).
LAST_RUN = None

_BUILD_CACHE = {}


def _build_device_kernel(cw, spread=True):
    """Build the SPMD Bass program for per-phase1-window chunk counts `cw`.

    Phase-1 windows have WP nodes (WN1=128 in bf16 mode, WN=256 in f32r
    mode); HALVES of them make up one 256-node MLP window.
    """
    C = int(sum(cw))
    WP = WN1 if PHASE1_BF16 else WN
    HALVES = WN // WP
    ADT = BF16 if PHASE1_BF16 else F32R  # attr + onehot dtype
    DDT = BF16 if PHASE1_BF16 else F32  # dst-index dtype

    nc = bass.Bass("TRN2")

    attr_d = nc.declare_dram_parameter("attr", [P, C * FC], ADT, isOutput=False)
    xt_d = nc.declare_dram_parameter("xt", [P, NPAD], F32R, isOutput=False)
    recip_d = nc.declare_dram_parameter("recip", [NPAD], F32, isOutput=False)
    w1_d = nc.declare_dram_parameter("w1", [2 * P, HID], F32R, isOutput=False)
    b1_d = nc.declare_dram_parameter("b1", [P, 2], F32, isOutput=False)
    w2_d = nc.declare_dram_parameter("w2", [2 * P, F_OUT], F32R, isOutput=False)
    b2_d = nc.declare_dram_parameter("b2", [P, 1], F32, isOutput=False)
    out_d = nc.declare_dram_parameter("outT", [P, NPAD], F32, isOutput=True)

    with tile.TileContext(nc) as tc:
        with (
            tc.tile_pool(name="const", bufs=1) as const,
            tc.tile_pool(name="attr", bufs=3) as attr_p,
            tc.tile_pool(name="oh", bufs=6) as oh_p,
            tc.tile_pool(name="oha", bufs=8) as oha_p,
            tc.tile_pool(name="abs", bufs=4) as abs_p,
            tc.tile_pool(name="xtw", bufs=3) as xt_p,
            tc.tile_pool(name="rcw", bufs=3) as rc_p,
            tc.tile_pool(name="agg", bufs=3) as agg_p,
            tc.tile_pool(name="hsb", bufs=4) as h_p,
            tc.tile_pool(name="osb", bufs=2) as o_p,
            tc.tile_pool(name="ps_s", bufs=3, space="PSUM") as ps_s,
            tc.tile_pool(name="ps_h", bufs=3, space="PSUM") as ps_h,
            tc.tile_pool(name="ps_o", bufs=2, space="PSUM") as ps_o,
        ):
            # --- constants ---
            iota_i = const.tile([P, WP], mybir.dt.int32)
            nc.gpsimd.iota(iota_i[:], pattern=[[1, WP]], base=0, channel_multiplier=0)
            iota_r = const.tile([P, KB, WP], DDT)  # 0..WP-1 per lane, x KB
            for j in range(KB):
                nc.vector.tensor_copy(out=iota_r[:, j, :], in_=iota_i[:])

            r_ap = recip_d[:]

            w1_sb = const.tile([P, 2, HID], F32R)  # [p, k, h]: W1[k*128+p, h]
            w2_sb = const.tile([P, 2, F_OUT], F32R)
            b1_sb = const.tile([P, 2], F32)
            b2_sb = const.tile([P, 1], F32)

            def load_weights():
                nc.sync.dma_start(
                    out=w1_sb[:], in_=w1_d[:].rearrange("(k p) h -> p k h", p=P)
                )
                nc.sync.dma_start(
                    out=w2_sb[:], in_=w2_d[:].rearrange("(k p) h -> p k h", p=P)
                )
                nc.sync.dma_start(out=b1_sb[:], in_=b1_d[:])
                nc.sync.dma_start(out=b2_sb[:], in_=b2_d[:])

            # --- per-MLP-window pipeline (MLP delayed one window) ---
            off = 0
            chunk_no = 0
            pending = []
            for w in range(list(range(NW))[-1] + 1 + 1):
                if w < NW:
                    pass
                else:
                    # flush: emit final window's MLP
                    w_m, agg, xt_w = pending.pop(0)
                    nsl = slice(w_m * WN, (w_m + 1) * WN)
                    h_sbs = []
                    for hi in range(2):
                        h_ps = ps_h.tile([P, WN], F32, tag="h")
                        hsl = slice(hi * P, (hi + 1) * P)
                        nc.tensor.matmul(
                            out=h_ps[:],
                            lhsT=w1_sb[:, 0, hsl],
                            rhs=xt_w[:],
                            start=True,
                            stop=False,
                        )
                        nc.tensor.matmul(
                            out=h_ps[:],
                            lhsT=w1_sb[:, 1, hsl],
                            rhs=agg[:],
                            start=False,
                            stop=True,
                        )
                        h_sb = h_p.tile([P, WN], F32R, tag="hsb")
                        nc.scalar.activation(
                            out=h_sb[:],
                            in_=h_ps[:],
                            func=mybir.ActivationFunctionType.Relu,
                            bias=b1_sb[:, hi : hi + 1],
                            scale=1.0,
                        )
                        h_sbs.append(h_sb)
                    o_ps = ps_o.tile([P, WN], F32, tag="o")
                    for k in range(2):
                        nc.tensor.matmul(
                            out=o_ps[:],
                            lhsT=w2_sb[:, k, :],
                            rhs=h_sbs[k][:],
                            start=(k == 0),
                            stop=(k == 1),
                        )
                    o_sb = o_p.tile([P, WN], F32, tag="osb")
                    nc.vector.tensor_scalar_add(
                        out=o_sb[:], in0=o_ps[:], scalar1=b2_sb[:, 0:1]
                    )
                    nc.sync.dma_start(out=out_d[:, nsl], in_=o_sb[:])
                    break
                nsl = slice(w * WN, (w + 1) * WN)
                agg = agg_p.tile([P, WN], F32R)

                xt_w = xt_p.tile([P, WN], F32R, tag="xtw")
                nc.sync.dma_start(out=xt_w[:], in_=xt_d[:, nsl])
                rc_w = rc_p.tile([P, WN], F32, tag="rcw")
                rc_src = bass.AP(
                    tensor=r_ap.tensor,
                    offset=r_ap.offset + w * WN,
                    ap=[[0, P], [1, WN]],
                )
                nc.sync.dma_start(out=rc_w[:], in_=rc_src)

                for hw in range(HALVES):
                    wp = w * HALVES + hw
                    cwn = cw[wp]
                    asl = slice(hw * WP, (hw + 1) * WP)
                    psl = slice(wp * WP, (wp + 1) * WP)
                    if cwn == 0:
                        nc.vector.memset(agg[:, asl], 0.0)
                        continue

                    a_t = attr_p.tile([P, cwn * FC], ADT, tag="attr")
                    nc.sync.dma_start(
                        out=a_t[:], in_=attr_d[:, off * FC : (off + cwn) * FC]
                    )
                    a_v = a_t[:].rearrange("p (c f) -> p c f", f=FC)

                    sums = ps_s.tile([P, WP], F32, tag="sums")
                    nact = int(cwn * ACT_FRAC)
                    for c in range(nact):
                        # onehot on the Scalar engine: relu(1 - |d - iota|)
                        t_t = abs_p.tile([P, WP], ADT, tag="abs")
                        nc.scalar.activation(
                            out=t_t[:],
                            in_=iota_r[:, 0, :],
                            func=mybir.ActivationFunctionType.Abs,
                            bias=a_v[:, c, F : F + 1],
                            scale=-1.0,
                        )
                        oh1 = oha_p.tile([P, WP], ADT, tag="oha")
                        nc.scalar.activation(
                            out=oh1[:],
                            in_=t_t[:],
                            func=mybir.ActivationFunctionType.Relu,
                            bias=1.0,
                            scale=-1.0,
                        )
                        nc.tensor.matmul(
                            out=sums[:],
                            lhsT=a_v[:, c, 0:F],
                            rhs=oh1[:],
                            start=(c == 0),
                            stop=(c == cwn - 1),
                        )
                    for c0 in range(nact, cwn, KB):
                        kb = min(KB, cwn - c0)
                        oh = oh_p.tile([P, KB, WP], ADT, tag="oh")
                        d_sl = a_v[:, c0 : c0 + kb, F : F + 1]
                        d_bc = bass.AP(
                            tensor=d_sl.tensor,
                            offset=d_sl.offset,
                            ap=list(d_sl.ap[:2]) + [[0, WP]],
                        )
                        nc.vector.tensor_tensor(
                            out=oh[:, :kb, :],
                            in0=iota_r[:, :kb, :],
                            in1=d_bc,
                            op=mybir.AluOpType.is_equal,
                        )
                        for j in range(kb):
                            c = c0 + j
                            nc.tensor.matmul(
                                out=sums[:],
                                lhsT=a_v[:, c, 0:F],
                                rhs=oh[:, j, :],
                                start=(c == 0),
                                stop=(c == cwn - 1),
                            )
                    # agg = sums / max(count, 1)
                    nc.vector.tensor_mul(
                        out=agg[:, asl], in0=sums[:], in1=rc_w[:, asl]
                    )
                    off += cwn

                if w == 0:
                    load_weights()
                pending.append((w, agg, xt_w))
                if len(pending) < 2:
                    continue
                w_m, agg, xt_w = pending.pop(0)
                nsl = slice(w_m * WN, (w_m + 1) * WN)

                # --- node MLP (feature-major, nodes on free dim) ---
                h_sbs = []
                for hi in range(2):
                    h_ps = ps_h.tile([P, WN], F32, tag="h")
                    hsl = slice(hi * P, (hi + 1) * P)
                    nc.tensor.matmul(
                        out=h_ps[:],
                        lhsT=w1_sb[:, 0, hsl],
                        rhs=xt_w[:],
                        start=True,
                        stop=False,
                    )
                    nc.tensor.matmul(
                        out=h_ps[:],
                        lhsT=w1_sb[:, 1, hsl],
                        rhs=agg[:],
                        start=False,
                        stop=True,
                    )
                    h_sb = h_p.tile([P, WN], F32R, tag="hsb")
                    nc.scalar.activation(
                        out=h_sb[:],
                        in_=h_ps[:],
                        func=mybir.ActivationFunctionType.Relu,
                        bias=b1_sb[:, hi : hi + 1],
                        scale=1.0,
                    )
                    h_sbs.append(h_sb)

                o_ps = ps_o.tile([P, WN], F32, tag="o")
                for k in range(2):
                    nc.tensor.matmul(
                        out=o_ps[:],
                        lhsT=w2_sb[:, k, :],
                        rhs=h_sbs[k][:],
                        start=(k == 0),
                        stop=(k == 1),
                    )
                o_sb = o_p.tile([P, WN], F32, tag="osb")
                nc.vector.tensor_scalar_add(
                    out=o_sb[:], in0=o_ps[:], scalar1=b2_sb[:, 0:1]
                )
                nc.sync.dma_start(out=out_d[:, nsl], in_=o_sb[:])

    if spread:
        _spread_waits(nc)
    return nc


def _preprocess(x, edge_index, edge_attr):
    """Sort edges by dst, bucket into per-core node-range shards, pad each
    phase-1 window's edge list to a multiple of 128 (chunk counts shared
    across cores so one SPMD program fits all)."""
    import ml_dtypes

    WP = WN1 if PHASE1_BF16 else WN
    NWP = NPAD // WP
    adt = ml_dtypes.bfloat16 if PHASE1_BF16 else np.float32

    dst = np.asarray(edge_index[1], dtype=np.int64)
    order = np.argsort(dst, kind="stable")
    ds = dst[order]

    # phase-1 window boundaries in the sorted edge array: [M, NWP+1]
    los = np.empty((M, NWP + 1), dtype=np.int64)
    for c in range(M):
        for w in range(NWP + 1):
            los[c, w] = c * NPC + min(w * WP, NPC)
    bounds = np.searchsorted(ds, los.ravel()).reshape(M, NWP + 1)
    L = bounds[:, 1:] - bounds[:, :-1]  # [M, NWP]
    cw = tuple(max(int(v), 1) for v in np.ceil(L.max(axis=0) / P).astype(np.int64))
    C = int(sum(cw))

    counts = np.bincount(dst, minlength=N_NODES).astype(np.float32)
    recip_full = 1.0 / np.maximum(counts, 1.0)

    per_core = []
    for c in range(M):
        idx = np.zeros(C * P, dtype=np.int64)
        dloc = np.full(C * P, -1.0, dtype=np.float32)
        pos = 0
        for w in range(NWP):
            s, e = int(bounds[c, w]), int(bounds[c, w + 1])
            n = e - s
            idx[pos : pos + n] = order[s:e]
            dloc[pos : pos + n] = (ds[s:e] - (c * NPC + w * WP)).astype(np.float32)
            pos += cw[w] * P
        attr = np.empty((C * P, F + 1), dtype=adt)  # last col = local dst idx
        attr[:, :F] = edge_attr[idx].astype(adt)
        attr[:, F] = dloc.astype(adt)
        attr_t = np.ascontiguousarray(
            attr.reshape(C, P, F + 1).transpose(1, 0, 2).reshape(P, C * (F + 1))
        )

        xt = np.zeros((P, NPAD), dtype=np.float32)
        xt[:, :NPC] = x[c * NPC : (c + 1) * NPC].T
        recip = np.zeros(NPAD, dtype=np.float32)
        recip[:NPC] = recip_full[c * NPC : (c + 1) * NPC]
        per_core.append({"attr": attr_t, "xt": xt, "recip": recip})
    return cw, per_core


def kernel(x, edge_index, edge_attr, W1, b1, W2, b2):
    global LAST_RUN
    x = np.ascontiguousarray(np.asarray(x, dtype=np.float32))
    edge_attr = np.ascontiguousarray(np.asarray(edge_attr, dtype=np.float32))
    W1 = np.ascontiguousarray(np.asarray(W1, dtype=np.float32))
    W2 = np.ascontiguousarray(np.asarray(W2, dtype=np.float32))
    b1c = np.ascontiguousarray(
        np.asarray(b1, dtype=np.float32).reshape(2, P).T
    )  # [128, 2]
    b2c = np.ascontiguousarray(np.asarray(b2, dtype=np.float32).reshape(P, 1))

    cw, per_core = _preprocess(x, edge_index, edge_attr)

    if cw not in _BUILD_CACHE:
        _BUILD_CACHE[cw] = _build_device_kernel(cw)
    nc = _BUILD_CACHE[cw]

    in_maps = []
    for c in range(M):
        pc = per_core[c]
        in_maps.append(
            {
                "attr": pc["attr"],
                "xt": pc["xt"],
                "recip": pc["recip"],
                "w1": W1,
                "b1": b1c,
                "w2": W2,
                "b2": b2c,
            }
        )

    trace = os.environ.get("KERNEL_TRACE") == "1"
    res = run_bass_kernel_spmd(nc, in_maps, core_ids=list(range(M)), trace=trace)
    LAST_RUN = res

    out = np.empty((N_NODES, F_OUT), dtype=np.float32)
    for c in range(M):
        out[c * NPC : (c + 1) * NPC] = res.results[c]["outT"][:, :NPC].T
    return out
